# revision 1
# baseline (speedup 1.0000x reference)
"""MixHop GNN (3 layers, hops {0,1,2}) on 8 Trainium2 NeuronCores.

Strategy (1D node partition):
 - Nodes padded to NPAD=100352 = 8*12544; core c owns destination rows
   [c*12544, (c+1)*12544).
 - gcn_norm edge weight w_e = dinv[src]*dinv[dst] is factorized:
   gather tables are pre-scaled by dinv[src] ("scale-in-table"), spmm
   output is post-scaled by dinv[dst] (per-partition ACT scale).
 - SpMM = batched dma_gather of source rows (bf16, 256B each) +
   selection-matrix matmul: for each 128-edge chunk, Sel[e,d] =
   (ldest[e]==d) built by one DVE is_equal in e-major layout
   [P, dest, chunk] so the broadcast operand is NOT the last AP dim
   (enables the DVE 2x_1p fast mode, ~2x cheaper); PE accumulates
   Sel_c.T @ G into the [128 dest x 128 feat] PSUM tile of its dest
   tile.
 - Edge layout: edges sort by (dest tile, src quarter); each
   (tile,quarter) bucket keeps at most K2=2 main chunks of 128 (the
   bucket mean is ~255, so a 3rd chunk would be ~2/3 padding); the
   ~2.5k overflow edges/core pool into KOV shared chunks per
   (7-tile gather group, quarter) and are routed to their dest tiles
   by group-wide f32 is_equal Sels (dest ids up to 895 are not bf16-
   exact).  This cuts gather slots from 150528 to 114688 (-24% gather
   DMA) at identical PE matmul count per tile (8 main + 4*KOV ov).
   The quarter split keeps int16 gather indices in range; slot counts
   are uniform across cores so one SPMD program serves all 8.
 - All psum post-scales / copies run on the otherwise-idle Activation
   engine; BN statistics are fused into those copies via accum_out
   (free-axis sum) + one ACT Square, so passes leave the DVE with
   nothing but Sel builds and never stall on stats.  The BN AllReduce
   is split: the hop0/hop1 half fires inside the preceding pass phase
   (collective cores idle there), only the hop2 half is chain-critical.
 - Layer 1 uses the commuted form out_j = (A^j x) W + (A^j 1) b: x is
   already replicated, so layer 1 needs only ONE AllGather.  u1/u2
   sinks emit y1T/y2T pre-transposed (feature-major) and the hop
   transforms stream them; hop1 runs under AG1, hop2 is interleaved
   into u2 by the Tile scheduler.  Emission order is scheduler
   PRIORITY: hop0 (and the xt load it needs) is emitted AFTER u1 so it
   back-fills idle slots and the AG1 window instead of displacing u1's
   early tile consumption (emitting it first cost ~20us of gather
   refill stall at pass start).
 - Layers 2/3 use a 3-single-pass schedule tuned so AllGathers overlap
   gather passes:  AGt2 -> (p1=A.t2 || AGt1) -> (p2=A.t1 || AGus) ->
   p3=A.us; the three AllGathers run back-to-back on the collective
   cores while p1/p2 hide beneath them.  The t2 transform sweep runs
   first after the BN AllReduce so AGt2 starts as early as possible;
   the j0/j1 sweep runs under AGt2.
 - Dense transforms are emitted NODE-major: psum[node,feat] +=
   h_b[:,q].T @ W[j,b] with the feature-major h block itself as lhsT,
   plus a K=1 ones-row matmul for the bias, so no PE transposes or
   extra DVE work appear anywhere in the transform sweeps; the dinv
   pre-scale of gather tables rides the per-partition ACT copy.
 - Phase boundaries are tc.no_sync_barrier() scheduler fences: they
   pin per-engine instruction order between phases (the list scheduler
   otherwise interleaves gather-blocked matmuls of pass N+1 ahead of
   transform work, head-of-line blocking every queue) while still
   letting independent engines overlap across the fence.
 - All matmul operands bf16 (fp32 PE matmul is 4x slower); PSUM
   accumulation fp32; final output written fp32.

Measured (TimelineSim instruction-cost model, full per-core program):
~3.22 ms vs 5.19 ms for the session-start baseline kernel (1.6x);
rel err 5.7e-3 on the 8-core hardware run.
"""
import os
import numpy as np
import ml_dtypes

import concourse.bass as bass
import concourse.mybir as mybir
import concourse.tile as tile
from concourse import library_config
from concourse.bass_utils import run_bass_kernel_spmd

bf16 = mybir.dt.bfloat16
f32 = mybir.dt.float32
i16 = mybir.dt.int16
BF = ml_dtypes.bfloat16

N = 100000
NC = 8
P = 128
D = 128
SHARD = 12544
NPAD = SHARD * NC
TPC = SHARD // P          # 98 dest tiles per core
GROUP = 7                 # dest tiles per gather group
NGROUPS = TPC // GROUP    # 7
GSZ = 512                 # transform moving-dim group
RQ = 25088                # gather source quarter-range rows (NPAD/4)
GBUFS = int(os.environ.get("KGBUFS", "2"))
EPS = 1e-5

AOP = mybir.AluOpType
AF = mybir.ActivationFunctionType


# ---------------------------------------------------------------- wait split
_SKIP_WAITSPLIT = (mybir.InstEventSemaphore,)


def _split_excess_waits(nc, keep=1):
    """This walrus build allows only 1 embedded sync-wait on most
    instructions; hoist extras into EventSemaphore insts placed before."""
    n = 0
    uid = [0]
    for fn in nc.m.functions:
        for blk in fn.blocks:
            insts = list(blk.instructions)
            out = []
            for inst in insts:
                si = inst.sync_info
                if (si is not None and si.on_wait and len(si.on_wait) > keep
                        and not isinstance(inst, _SKIP_WAITSPLIT)):
                    waits = list(si.on_wait)
                    extra, rest = waits[:-keep], waits[-keep:]
                    for w in extra:
                        uid[0] += 1
                        out.append(mybir.InstEventSemaphore(
                            name=f"evws_{uid[0]}",
                            engine=inst.engine,
                            ins=[], outs=[],
                            sync_info=mybir.SyncInfo(on_wait=[w], on_update=[]),
                        ))
                        n += 1
                    inst.sync_info = mybir.SyncInfo(
                        on_wait=rest, on_update=list(si.on_update or []))
                out.append(inst)
            if len(out) != len(insts):
                blk.instructions = out
    return n


# ---------------------------------------------------------------- host prep
def _host_prep(x, edge_index):
    row = edge_index[0].astype(np.int64)
    col = edge_index[1].astype(np.int64)
    deg = np.bincount(col, minlength=N).astype(np.float64)
    dinv = np.where(deg > 0, 1.0 / np.sqrt(np.maximum(deg, 1.0)), 0.0)
    s1 = dinv * np.bincount(col, weights=dinv[row], minlength=N)
    s2 = dinv * np.bincount(col, weights=dinv[row] * s1[row], minlength=N)

    dinv_pad = np.zeros(NPAD, np.float64)
    dinv_pad[:N] = dinv
    x_pad = np.zeros((NPAD, D), np.float32)
    x_pad[:N] = x
    xtbl = (dinv_pad[:, None] * x_pad).astype(BF)      # L1 gather table
    s1_pad = np.zeros(NPAD, np.float32)
    s1_pad[:N] = s1
    s2_pad = np.zeros(NPAD, np.float32)
    s2_pad[:N] = s2
    ones_pad = np.zeros(NPAD, np.float32)
    ones_pad[:N] = 1.0

    # per-core edge structures: each (tile, quarter) bucket is capped at
    # K2*128 main slots; the rest ("overflow", ~2.5k edges/core) is pooled
    # per (gather group, quarter) into KOV shared chunks whose edges are
    # routed to their dest tiles by group-wide selection matrices.
    K2 = 2
    CPT = 4 * K2
    pre = []
    maxov = 0
    for c in range(NC):
        lo, hi = c * SHARD, (c + 1) * SHARD
        m = (col >= lo) & (col < hi)
        r, cc = row[m], col[m] - lo
        t = cc >> 7
        res = r // RQ
        key = t * 4 + res
        order = np.argsort(key, kind="stable")
        r_s, cc_s, key_s = r[order], cc[order], key[order]
        cnt = np.bincount(key_s, minlength=TPC * 4)
        starts = np.zeros(TPC * 4, np.int64)
        starts[1:] = np.cumsum(cnt)[:-1]
        rank = np.arange(len(key_s)) - starts[key_s]
        ismain = rank < K2 * P
        ovg = (np.minimum(cnt, 10 ** 9) - np.minimum(cnt, K2 * P))
        ovg = ovg.reshape(TPC, 4).reshape(NGROUPS, GROUP, 4).sum(axis=1)
        maxov = max(maxov, int(ovg.max()))
        pre.append((r_s, cc_s, key_s, rank, ismain))

    KOV = max(1, -(-maxov // P))
    SEG_MAIN = GROUP * K2 * P
    SEGIDX = SEG_MAIN + KOV * P
    IDXTOT = NGROUPS * 4 * SEGIDX

    cores = []
    for c in range(NC):
        r_s, cc_s, key_s, rank, ismain = pre[c]
        tt_, rr_ = key_s // 4, key_s % 4
        idx_main = np.zeros((TPC, 4, K2 * P), np.int16)
        ld_main = np.full((TPC, 4, K2 * P), -1.0, np.float32)
        sel = ismain
        idx_main[tt_[sel], rr_[sel], rank[sel]] = (r_s[sel] % RQ).astype(
            np.int16)
        ld_main[tt_[sel], rr_[sel], rank[sel]] = (cc_s[sel] & 127).astype(
            np.float32)

        ov = ~ismain
        g_, q_ = tt_[ov] // GROUP, rr_[ov]
        okey = g_ * 4 + q_
        oorder = np.argsort(okey, kind="stable")
        okey_s = okey[oorder]
        ocnt = np.bincount(okey_s, minlength=NGROUPS * 4)
        ostarts = np.zeros(NGROUPS * 4, np.int64)
        ostarts[1:] = np.cumsum(ocnt)[:-1]
        orank = np.arange(len(okey_s)) - ostarts[okey_s]
        assert orank.max(initial=0) < KOV * P
        rov, ccov = r_s[ov][oorder], cc_s[ov][oorder]
        idx_ov = np.zeros((NGROUPS, 4, KOV * P), np.int16)
        ld_ov = np.full((NGROUPS, 4, KOV * P), -1.0, np.float32)
        idx_ov[g_[oorder], q_[oorder], orank] = (rov % RQ).astype(np.int16)
        ld_ov[g_[oorder], q_[oorder], orank] = (
            ccov - g_[oorder] * GROUP * P).astype(np.float32)

        idx16 = np.zeros((P, IDXTOT // 16), np.int16)
        segc = SEGIDX // 16
        for g in range(NGROUPS):
            for q in range(4):
                seg = np.concatenate(
                    [idx_main[g * GROUP:(g + 1) * GROUP, q, :].ravel(),
                     idx_ov[g, q, :]])
                blk = seg.reshape(-1, 16).T          # [16, SEGIDX/16]
                i0 = (g * 4 + q) * segc
                idx16[:, i0:i0 + segc] = np.tile(blk, (8, 1))

        # per tile, chunk index cidx = quarter*K2 + k
        ldest = ld_main.reshape(TPC * CPT, P).T.astype(BF)   # [P, NCHUNK]
        ldov = ld_ov.reshape(NGROUPS * 4 * KOV, P).T.astype(np.float32)
        lo, hi = c * SHARD, (c + 1) * SHARD
        d1 = dinv_pad[lo:hi].reshape(TPC, P).T.astype(np.float32)
        d2 = (dinv_pad[lo:hi] ** 2).reshape(TPC, P).T.astype(np.float32)
        xt = x_pad[lo:hi].T.astype(BF)                   # [P, SHARD]
        srow = np.concatenate([s1_pad[lo:hi], s2_pad[lo:hi]])[None, :]
        cores.append(dict(idx16=idx16, ldest=ldest, ldov=ldov, d1=d1, d2=d2,
                          xt=xt, srow=srow.astype(BF)))
    return xtbl, cores, KOV


def _pack_consts(core, KOV, W0, b0, W1, b1, W2, b2, bn_g, bn_b):
    CPT = 8                       # 4 quarters x K2(2) main chunks per tile
    NCHUNK = TPC * CPT
    # e-major iota: iota2[p, e*CPT + c] = e  (c is the last, packed dim)
    iota2 = np.repeat(np.arange(P, dtype=np.float32), CPT)[None, :]
    iota2 = np.tile(iota2, (P, 1)).astype(BF)            # [P, P*CPT]
    # group-wide e-major iota for overflow chunks: values 0..GROUP*P-1.
    # Kept in f32 (cf): dest ids up to 895 are NOT exactly representable
    # in bf16, so the overflow is_equal must compare in f32.
    iota2g = np.repeat(np.arange(GROUP * P, dtype=np.float32), KOV)[None, :]
    iota2g = np.tile(iota2g, (P, 1)).astype(np.float32)  # [P, GROUP*P*KOV]
    ident = np.eye(P, dtype=np.float32)
    w0c = np.concatenate([W0[j] for j in range(3)], axis=1)          # [P,3P]
    blocks = []
    for W in (W1, W2):
        for j in range(3):
            for b in range(3):
                blocks.append(W[j][b * P:(b + 1) * P, :])
    w12c = np.concatenate(blocks, axis=1)                            # [P,18P]
    cb = np.concatenate([iota2, core["ldest"], ident.astype(BF),
                         w0c.astype(BF), w12c.astype(BF)], axis=1)
    # rows: s1 | s2 (streamed [1,w] slices), then resident bias rows
    # [9P] and a ones row [P] (lhsT of K=1 bias matmuls)
    br = np.concatenate([b0.reshape(1, -1), b1.reshape(1, -1),
                         b2.reshape(1, -1)], axis=1)                 # [1,9P]
    cr = np.concatenate([core["srow"].astype(np.float32),
                         br.astype(np.float32),
                         np.ones((1, P), np.float32)], axis=1).astype(BF)
    # f32: dinv1 | dinv2 | bng(6) | bnb(6) | identf32(128) | bias cols (9)
    bng = np.stack([bn_g[l].reshape(3, P).T for l in range(2)], axis=0)
    bnb = np.stack([bn_b[l].reshape(3, P).T for l in range(2)], axis=0)
    bcol = np.stack([b0[0], b0[1], b0[2], b1[0], b1[1], b1[2],
                     b2[0], b2[1], b2[2]], axis=1)        # [P, 9]
    cf = np.concatenate([core["d1"], core["d2"],
                         bng[0], bng[1], bnb[0], bnb[1], ident, bcol,
                         iota2g, core["ldov"].astype(np.float32)], axis=1)
    return cb.astype(BF), cr, cf.astype(np.float32)


# ---------------------------------------------------------------- device
def _build(KOV, phase="full", for_sim=False):
    K2 = 2
    CPT = 4 * K2
    NCHUNK = TPC * CPT
    NOVC = NGROUPS * 4 * KOV          # overflow chunk count
    SEGIDX = GROUP * K2 * P + KOV * P
    IDXTOT = NGROUPS * 4 * SEGIDX
    SEGC = SEGIDX // 16
    NCB = CPT * P + NCHUNK + P + 3 * P + 18 * P
    NCR = 2 * SHARD + 10 * P
    NCF = TPC * 2 + 12 + P + 9 + GROUP * P * KOV + NGROUPS * 4 * KOV
    NG = (SHARD + GSZ - 1) // GSZ      # transform groups (25)

    nc = bass.Bass(num_devices=NC)
    xtbl_t = nc.dram_tensor("xtbl", [NPAD, P], bf16, kind="ExternalInput")
    xt_t = nc.dram_tensor("xt", [P, SHARD], bf16, kind="ExternalInput")
    idx_t = nc.dram_tensor("idx", [P, IDXTOT // 16], i16, kind="ExternalInput")
    cb_t = nc.dram_tensor("cb", [P, NCB], bf16, kind="ExternalInput")
    cr_t = nc.dram_tensor("cr", [1, NCR], bf16, kind="ExternalInput")
    cf_t = nc.dram_tensor("cf", [P, NCF], f32, kind="ExternalInput")
    out_t = nc.dram_tensor("out", [SHARD, 3 * P], f32, kind="ExternalOutput")

    with tile.TileContext(nc) as tc:
        with (
            tc.tile_pool(name="const", bufs=1) as cpool,
            tc.tile_pool(name="gath", bufs=2) as gpool,
            tc.tile_pool(name="sel", bufs=int(os.environ.get("KSELB", "3")) ) as spool,
            tc.tile_pool(name="work", bufs=3) as wpool,
            tc.tile_pool(name="hb", bufs=int(os.environ.get("KHB", "4")) ) as hpool,
            tc.tile_pool(name="stg", bufs=2) as stgpool,
            tc.tile_pool(name="stat", bufs=1) as stpool,
            tc.tile_pool(name="psp", bufs=int(os.environ.get("KSPB", "2")), space="PSUM") as ps_sp,
            tc.tile_pool(name="ptp", bufs=int(os.environ.get("KTPB", "2")), space="PSUM") as ps_tp,
            tc.tile_pool(name="ptf", bufs=int(os.environ.get("KTFB", "4")), space="PSUM") as ps_tf,
            tc.tile_pool(name="dram", bufs=1, space="DRAM") as dpool,
        ):
            nc.gpsimd.load_library(library_config.mlp)

            idx_sb = cpool.tile([P, IDXTOT // 16], i16)
            nc.sync.dma_start(idx_sb[:], idx_t[:])
            cb_sb = cpool.tile([P, NCB], bf16)
            nc.sync.dma_start(cb_sb[:], cb_t[:])
            cf_sb = cpool.tile([P, NCF], f32)
            nc.sync.dma_start(cf_sb[:], cf_t[:])
            xt_sb = cpool.tile([P, SHARD], bf16)   # loaded later (see L1)

            o = 0
            iota2_sb = cb_sb[:, o:o + CPT * P]; o += CPT * P
            ldest_sb = cb_sb[:, o:o + NCHUNK]; o += NCHUNK
            ident_sb = cb_sb[:, o:o + P]; o += P
            of0 = 2 * TPC + 12 + P + 9
            iota2g_sb = cf_sb[:, of0:of0 + GROUP * P * KOV]
            ldov_sb = cf_sb[:, of0 + GROUP * P * KOV:
                            of0 + GROUP * P * KOV + NOVC]
            w0_sb = cb_sb[:, o:o + 3 * P]; o += 3 * P
            w12_sb = cb_sb[:, o:o + 18 * P]

            iota3 = iota2_sb.rearrange("p (e c) -> p e c", c=CPT)
            iota3g = iota2g_sb.rearrange("p (e c) -> p e c", c=KOV)

            def wblk(l, j, b):  # layer l in {1,2}
                i = ((l - 1) * 9 + j * 3 + b) * P
                return w12_sb[:, i:i + P]

            def bcol(l, j):     # bias column [P,1] f32, layer l in {0,1,2}
                i = 2 * TPC + 12 + P + l * 3 + j
                return cf_sb[:, i:i + 1]

            def load_srow(which, n0, w, tag):
                # which: 0=s1, 1=s2; stream [1,w] slice from DRAM
                i = which * SHARD + n0
                rt = wpool.tile([1, GSZ], bf16, name="row", tag=tag)
                nc.sync.dma_start(rt[:1, :w], cr_t[:1, i:i + w])
                return rt[:1, :w]

            dinv1_sb = cf_sb[:, 0:TPC]
            dinv2_sb = cf_sb[:, TPC:2 * TPC]

            def bng(l, b):
                return cf_sb[:, 2 * TPC + l * 3 + b:2 * TPC + l * 3 + b + 1]

            def bnb(l, b):
                i = 2 * TPC + 6 + l * 3 + b
                return cf_sb[:, i:i + 1]

            identf_sb = cf_sb[:, 2 * TPC + 12:2 * TPC + 12 + P]
            # (ones row lives at rbias_sb[:1, 9P:10P])
            rbias_sb = cpool.tile([1, 10 * P], bf16)  # bias rows [9P] | ones
            nc.sync.dma_start(rbias_sb[:], cr_t[:1, 2 * SHARD:])

            def brow(l, j):     # [1,P] bias row, layer l in {0,1,2}
                i = (l * 3 + j) * P
                return rbias_sb[:1, i:i + P]

            ones_row = None     # set below: rbias_sb[:1, 9P:10P]

            def srow_b(j):      # [1,P] lhsT row of b0[j], j in {1,2}
                return brow(0, j)

            # DRAM staging
            y1T = dpool.tile([P, SHARD], bf16, name="y1T")
            y2T = dpool.tile([P, SHARD], bf16, name="y2T")
            rawh = {}
            for l in (1, 2):
                for b in range(3):
                    rawh[(l, b)] = dpool.tile([P, SHARD], bf16,
                                              name=f"rawh{l}{b}")
            ag_t1 = dpool.tile([SHARD, P], bf16, name="ag_t1", bufs=1)
            ag_t2 = dpool.tile([SHARD, P], bf16, name="ag_t2", bufs=1)
            ag_us = dpool.tile([SHARD, P], bf16, name="ag_us", bufs=1)
            _tbl_n = [0]

            def new_tbl():
                _tbl_n[0] += 1
                return dpool.tile([NPAD, P], bf16, name=f"tbl{_tbl_n[0]}",
                                  bufs=1, addr_space="Shared")
            arin = [dpool.tile([P, 8], f32, name=f"arin{l}") for l in range(4)]
            arout = [dpool.tile([P, 8], f32, name=f"arout{l}",
                                addr_space="Shared") for l in range(4)]
            part_d = dpool.tile([SHARD, P], bf16, name="part_d")

            def quarter_views(tensor_ap):
                return [tensor_ap[q * RQ:(q + 1) * RQ, :] for q in range(4)]

            segreg = nc.gpsimd.to_reg(SEGIDX)

            # -------------------------------------------------- spmm unit
            # sink(t, ps) consumes the [dest 128 x feat 128] fp32 PSUM tile
            # of dest tile t.
            def spmm_unit(tbl_ap, sink, uname, aux=None):
                # aux: optional list of thunks; spread across gather groups
                # (emitted after each group's tiles) so their engine work
                # interleaves into the queues instead of head-of-line
                # blocking behind / in front of the pass.
                tv = quarter_views(tbl_ap)
                deferred = []
                DEFER = int(os.environ.get("KDEFER", "0"))
                for g in range(NGROUPS):
                    gt = []
                    for res in range(4):
                        gtile = gpool.tile([P, SEGIDX], bf16,
                                           name=f"g{uname}", tag=f"g{res}",
                                           bufs=GBUFS)
                        seg = (g * 4 + res) * SEGC
                        if os.environ.get("KNOGATHER"):
                            nc.vector.memset(gtile[:], 0.25)
                        else:
                            nc.gpsimd.dma_gather(
                                out_ap=gtile.rearrange("p (c e) -> p c e", e=P),
                                in_ap=tv[res],
                                idxs_ap=idx_sb[:, seg:seg + SEGC],
                                num_idxs=SEGIDX,
                                num_idxs_reg=segreg,
                                elem_size=P,
                                single_packet=False,
                            )
                        gt.append(gtile.rearrange("p (c e) -> p c e", e=P))
                    # group-wide overflow sels: sov[p, e, c] = (ldov==e),
                    # e over the whole group's GROUP*P dests
                    sov = []
                    for res in range(4):
                        so = spool.tile([P, GROUP * P * KOV], bf16,
                                        name=f"so{uname}", tag=f"so{res}",
                                        bufs=2)
                        # f32 compare (bf16 cannot represent ids > 256
                        # exactly); bf16 output feeds the PE directly
                        so3 = so.rearrange("p (e c) -> p e c", c=KOV)
                        i0 = (g * 4 + res) * KOV
                        nc.vector.tensor_tensor(
                            out=so3,
                            in0=iota3g,
                            in1=ldov_sb[:, None, i0:i0 + KOV]
                                .to_broadcast([P, GROUP * P, KOV]),
                            op=AOP.is_equal)
                        sov.append(so3)
                    for tt in range(GROUP):
                        t = g * GROUP + tt
                        # Sel in e-major layout: sel[p, e, c] = (ldest[p,c]==e)
                        # -> last AP dim (c) is packed for all operands, so
                        # the DVE 2x_1p fast mode applies.
                        sel = spool.tile([P, P * CPT], bf16, name=f"s{uname}",
                                         tag="sel")
                        sel3 = sel.rearrange("p (e c) -> p e c", c=CPT)
                        nc.vector.tensor_tensor(
                            out=sel3,
                            in0=iota3,
                            in1=ldest_sb[:, None, t * CPT:(t + 1) * CPT]
                                .to_broadcast([P, P, CPT]),
                            op=AOP.is_equal)
                        ps = ps_sp.tile([P, P], f32, name=f"p{uname}",
                                        tag="sp", space="PSUM")
                        for cidx in range(CPT):
                            res, k = divmod(cidx, K2)
                            nc.tensor.matmul(
                                ps[:],
                                lhsT=sel3[:, :, cidx],
                                rhs=gt[res][:, tt * K2 + k, :],
                                start=(cidx == 0), stop=False)
                        # overflow contributions: this tile's rows of the
                        # group-wide sels pick its edges out of the shared
                        # overflow chunks
                        nov = 4 * KOV
                        i = 0
                        for res in range(4):
                            for c2 in range(KOV):
                                i += 1
                                nc.tensor.matmul(
                                    ps[:],
                                    lhsT=sov[res][:, tt * P:(tt + 1) * P, c2],
                                    rhs=gt[res][:, GROUP * K2 + c2, :],
                                    start=False, stop=(i == nov))
                        d = sink(t, ps)
                        if d is not None:
                            deferred.append(d)
                        while len(deferred) > DEFER:
                            deferred.pop(0)()
                    if aux:
                        lo = (len(aux) * g) // NGROUPS
                        hi = (len(aux) * (g + 1)) // NGROUPS
                        for fn in aux[lo:hi]:
                            fn()
                for fn in deferred:
                    fn()

            # sinks ------------------------------------------------------
            # All psum post-scales run on the Activation engine (ACT is
            # otherwise idle; keeps DVE free for sel builds).
            def act_scale(ps, scale_col, dtype, tag):
                a = wpool.tile([P, P], dtype, name=f"as{tag}", tag=tag,
                               bufs=6)
                nc.scalar.activation(out=a[:], in_=ps[:], func=AF.Copy,
                                     scale=scale_col)
                return a

            def mk_sink_tblwr(dst, which):
                # which: 1 -> dinv1 (h value), 2 -> dinv2 (next gather table)
                dv = dinv1_sb if which == 1 else dinv2_sb
                def sink(t, ps):
                    a = act_scale(ps, dv[:, t:t + 1], bf16, f"tw{which}")
                    nc.sync.dma_start(dst[t * P:(t + 1) * P, :], a[:])
                return sink

            def hT_stage2(a, t, dst_hT, stS=None, stQ=None, rawdst=None):
                def fin():
                    pst = ps_tp.tile([P, P], bf16, name="ptd", tag="tp",
                                     space="PSUM")
                    nc.tensor.transpose(pst[:], a[:], ident_sb)
                    c = wpool.tile([P, P], bf16, name="chT", tag="chT")
                    if stS is None:
                        nc.scalar.activation(out=c[:], in_=pst[:],
                                             func=AF.Copy)
                    else:
                        nc.scalar.activation(out=c[:], in_=pst[:],
                                             func=AF.Copy,
                                             accum_out=stS[:, t:t + 1])
                    dst = rawdst if rawdst is not None else dst_hT
                    nc.sync.dma_start(dst[:, t * P:(t + 1) * P], c[:])
                    if stQ is not None:
                        sq = wpool.tile([P, P], bf16, name="sq", tag="sq")
                        nc.scalar.activation(out=sq[:], in_=c[:],
                                             func=AF.Square,
                                             accum_out=stQ[:, t:t + 1])
                return fin

            def mk_sink_dual(dst_hT, dst_tbl):
                # h-value (dinv1), transposed to feature-major -> dst_hT;
                # table value (dinv2) -> dst_tbl rows (AllGather input).
                def sink(t, ps):
                    a = act_scale(ps, dinv1_sb[:, t:t + 1], bf16, "sh")
                    b = act_scale(ps, dinv2_sb[:, t:t + 1], bf16, "st")
                    nc.sync.dma_start(dst_tbl[t * P:(t + 1) * P, :], b[:])
                    return hT_stage2(a, t, dst_hT)
                return sink

            def mk_sink_hT(dst_hT):
                # h-value transposed to feature-major only.
                def sink(t, ps):
                    a = act_scale(ps, dinv1_sb[:, t:t + 1], bf16, "sh")
                    return hT_stage2(a, t, dst_hT)
                return sink

            def mk_sink_block(l, b, stS, stQ):
                # out_j = dinv1*psum -> transpose -> stats + rawh[l][b]
                def sink(t, ps):
                    a = act_scale(ps, dinv1_sb[:, t:t + 1], bf16, "sh")
                    return hT_stage2(a, t, None, stS=stS, stQ=stQ,
                                     rawdst=rawh[(l, b)])
                return sink

            def mk_sink_out(colbase):
                def sink(t, ps):
                    a = act_scale(ps, dinv1_sb[:, t:t + 1], f32, "so")
                    nc.sync.dma_start(
                        out_t[t * P:(t + 1) * P, colbase:colbase + P], a[:])
                return sink

            def allgather(src):
                dst = new_tbl()
                nc.gpsimd.collective_compute(
                    "AllGather", AOP.bypass,
                    replica_groups=[list(range(NC))],
                    ins=[src[:]], outs=[dst[:]])
                return dst

            # debug sink: write dinv*psum as f32 straight to OUT cols 0:128
            def mk_sink_dbg(colbase):
                def sink(t, ps):
                    a = act_scale(ps, dinv1_sb[:, t:t + 1], f32, "so")
                    nc.sync.dma_start(
                        out_t[t * P:(t + 1) * P, colbase:colbase + P], a[:])
                return sink

            # BN stats reduce + A/B -----------------------------------------
            st = {}

            def bn_ar(aidx, cols):
                """AllReduce the given [(S,Q)] stat tiles; returns g [P,8]
                with S-sums in cols 0..k-1 and Q-sums in cols 4..4+k-1."""
                ar = wpool.tile([P, 8], f32, name=f"arr{aidx}", tag="ar")
                for i, (S, Q) in enumerate(cols):
                    nc.vector.reduce_sum(out=ar[:, i:i + 1], in_=S[:],
                                         axis=mybir.AxisListType.X)
                    nc.vector.reduce_sum(out=ar[:, 4 + i:5 + i], in_=Q[:],
                                         axis=mybir.AxisListType.X)
                nc.sync.dma_start(arin[aidx][:], ar[:])
                nc.gpsimd.collective_compute(
                    "AllReduce", AOP.add, replica_groups=[list(range(NC))],
                    ins=[arin[aidx][:]], outs=[arout[aidx][:]])
                g = wpool.tile([P, 8], f32, name=f"arg{aidx}", tag="ar")
                nc.sync.dma_start(g[:], arout[aidx][:])
                return g

            def ab_cols(g, l, bs, A, B, tag):
                """A/B columns for blocks bs from AR result g."""
                k = len(bs)
                mu = wpool.tile([P, 2], f32, name=f"mu{tag}", tag="mu")
                va = wpool.tile([P, 2], f32, name=f"va{tag}", tag="mu")
                nc.vector.tensor_scalar(out=mu[:, :k], in0=g[:, 0:k],
                                        scalar1=1.0 / N, scalar2=None,
                                        op0=AOP.mult)
                nc.vector.tensor_scalar(out=va[:, :k], in0=g[:, 4:4 + k],
                                        scalar1=1.0 / N, scalar2=None,
                                        op0=AOP.mult)
                musq = wpool.tile([P, 2], f32, name=f"ms{tag}", tag="mu")
                nc.vector.tensor_tensor(out=musq[:, :k], in0=mu[:, :k],
                                        in1=mu[:, :k], op=AOP.mult)
                nc.vector.tensor_tensor(out=va[:, :k], in0=va[:, :k],
                                        in1=musq[:, :k], op=AOP.subtract)
                ve = wpool.tile([P, 2], f32, name=f"ve{tag}", tag="mu")
                nc.vector.tensor_scalar(out=ve[:, :k], in0=va[:, :k],
                                        scalar1=float(EPS), scalar2=None,
                                        op0=AOP.add)
                sq_ = wpool.tile([P, 2], f32, name=f"sv{tag}", tag="mu")
                nc.scalar.activation(out=sq_[:, :k], in_=ve[:, :k],
                                     func=AF.Sqrt)
                rs = wpool.tile([P, 2], f32, name=f"rg{tag}", tag="mu")
                nc.vector.reciprocal(out=rs[:, :k], in_=sq_[:, :k])
                muA = wpool.tile([P, 2], f32, name=f"ma{tag}", tag="mu")
                for i, b in enumerate(bs):
                    nc.vector.tensor_tensor(out=A[:, b:b + 1],
                                            in0=rs[:, i:i + 1],
                                            in1=bng(l, b), op=AOP.mult)
                    nc.vector.tensor_tensor(out=muA[:, i:i + 1],
                                            in0=mu[:, i:i + 1],
                                            in1=A[:, b:b + 1], op=AOP.mult)
                    nc.vector.tensor_tensor(out=B[:, b:b + 1],
                                            in0=bnb(l, b),
                                            in1=muA[:, i:i + 1],
                                            op=AOP.subtract)

            # transform helpers ---------------------------------------------
            def tf_stats_raw(ps, w, grp, stS, stQ, raw_dst, n0, bias=None):
                """psum [P,w] -> bf16 copy (+per-feature bias) -> raw_dst.
                S/Q stats fused into ACT ops via accum_out (no DVE work, so
                concurrent spmm sel builds never queue behind stats)."""
                cp = hpool.tile([P, GSZ], bf16, name="cpt", tag="cpt")
                if bias is None:
                    nc.scalar.activation(out=cp[:, :w], in_=ps[:, :w],
                                         func=AF.Copy,
                                         accum_out=stS[:, grp:grp + 1])
                else:
                    nc.scalar.activation(out=cp[:, :w], in_=ps[:, :w],
                                         func=AF.Identity, bias=bias,
                                         accum_out=stS[:, grp:grp + 1])
                scr = wpool.tile([P, GSZ], bf16, name="sqt", tag="sqt")
                nc.scalar.activation(out=scr[:, :w], in_=cp[:, :w],
                                     func=AF.Square,
                                     accum_out=stQ[:, grp:grp + 1])
                nc.sync.dma_start(raw_dst[:, n0:n0 + w], cp[:, :w])

            def nm_tblock(l, j, hb, n0, w, dst):
                """Node-major t-block: one [P,GSZ] psum per group; each
                128-node quarter q accumulates sum_b hb[b][:,q].T @ W[l][j,b]
                + ones.T @ brow into its sub-region, then one ACT Copy with
                per-partition dinv scale -> bf16 -> dest-major agbuf rows."""
                ones = rbias_sb[:1, 9 * P:10 * P]
                nq = w // P
                ps = ps_tf.tile([P, GSZ], f32, name="nmb", tag="tf",
                                space="PSUM")
                for q in range(nq):
                    c0 = q * P
                    for b in range(3):
                        nc.tensor.matmul(ps[:, c0:c0 + P],
                                         lhsT=hb[b][:, c0:c0 + P],
                                         rhs=wblk(l, j, b),
                                         start=(b == 0), stop=False)
                    nc.tensor.matmul(ps[:, c0:c0 + P], lhsT=ones,
                                     rhs=brow(l, j), start=False, stop=True)
                for q in range(nq):
                    c0 = q * P
                    tt = (n0 + c0) // P
                    ob = wpool.tile([P, P], bf16, name="ob", tag="ob",
                                    bufs=6)
                    nc.scalar.activation(out=ob[:], in_=ps[:, c0:c0 + P],
                                         func=AF.Copy,
                                         scale=dinv1_sb[:, tt:tt + 1])
                    nc.sync.dma_start(dst[n0 + c0:n0 + c0 + P, :], ob[:])

            def nm_out0(l, hb, n0, w):
                """Final hop0 block node-major straight to OUT[:, 0:P]."""
                ones = rbias_sb[:1, 9 * P:10 * P]
                nq = w // P
                ps = ps_tf.tile([P, GSZ], f32, name="nmo", tag="tf",
                                space="PSUM")
                for q in range(nq):
                    c0 = q * P
                    for b in range(3):
                        nc.tensor.matmul(ps[:, c0:c0 + P],
                                         lhsT=hb[b][:, c0:c0 + P],
                                         rhs=wblk(l, 0, b),
                                         start=(b == 0), stop=False)
                    nc.tensor.matmul(ps[:, c0:c0 + P], lhsT=ones,
                                     rhs=brow(l, 0), start=False, stop=True)
                for q in range(nq):
                    c0 = q * P
                    of = wpool.tile([P, P], f32, name="of", tag="of",
                                    bufs=6)
                    nc.scalar.activation(out=of[:], in_=ps[:, c0:c0 + P],
                                         func=AF.Copy)
                    nc.sync.dma_start(
                        out_t[n0 + c0:n0 + c0 + P, 0:P], of[:])

            import contextlib

            @contextlib.contextmanager
            def tier():
                # scheduler-only fence: later phases' instructions may not
                # be scheduled ahead of earlier phases' in any engine queue
                tc.no_sync_barrier()
                yield

            if phase == "u1":
                spmm_unit(xtbl_t[:], mk_sink_dbg(0), "u1")
            elif phase == "u2":
                spmm_unit(xtbl_t[:], mk_sink_dual(y1T, ag_t2), "u1")
                tbx = allgather(ag_t2)
                spmm_unit(tbx[:], mk_sink_dbg(0), "u2")
            if phase == "full":
                # ============================================== LAYER 1
                for key in ("S0", "Q0", "S1", "Q1", "S2", "Q2"):
                    st[(1, key)] = stpool.tile([P, NG], f32, name=f"st1{key}")

                def hop0_grp(grp):
                    n0 = grp * GSZ
                    w = min(GSZ, SHARD - n0)
                    ps0 = ps_tf.tile([P, GSZ], f32, name="tf0", tag="tf",
                                     space="PSUM")
                    nc.tensor.matmul(ps0[:, :w], lhsT=w0_sb[:, 0:P],
                                     rhs=xt_sb[:, n0:n0 + w], start=True,
                                     stop=True)
                    tf_stats_raw(ps0, w, grp, st[(1, "S0")],
                                 st[(1, "Q0")], rawh[(1, 0)], n0,
                                 bias=bcol(0, 0))

                def hop12_grp(hop, grp):
                    n0 = grp * GSZ
                    w = min(GSZ, SHARD - n0)
                    ysrc = y1T if hop == 1 else y2T
                    yg = hpool.tile([P, GSZ], bf16, name="yg", tag="yg")
                    nc.sync.dma_start(yg[:, :w], ysrc[:, n0:n0 + w])
                    ps1 = ps_tf.tile([P, GSZ], f32, name="tfh", tag="tf",
                                     space="PSUM")
                    nc.tensor.matmul(ps1[:, :w],
                                     lhsT=w0_sb[:, hop * P:(hop + 1) * P],
                                     rhs=yg[:, :w], start=True, stop=False)
                    nc.tensor.matmul(ps1[:, :w], lhsT=srow_b(hop),
                                     rhs=load_srow(hop - 1, n0, w, "rs"),
                                     start=False, stop=True)
                    tf_stats_raw(ps1, w, grp, st[(1, f"S{hop}")],
                                 st[(1, f"Q{hop}")], rawh[(1, hop)], n0)

                # u1: A.x -> y1T (feature-major) + u2 gather table (dinv2)
                spmm_unit(xtbl_t[:], mk_sink_dual(y1T, ag_t2), "u1")
                # hop0 emitted AFTER u1 (same phase): lower scheduler
                # priority, so it back-fills idle PE/ACT slots and the AG1
                # window instead of displacing u1's early tile consumption.
                # xt (3.2MB, only needed here) is also loaded at this
                # priority so it never competes with u1's opening gathers.
                nc.sync.dma_start(xt_sb[:], xt_t[:])
                for g in range(NG):
                    hop0_grp(g)
                tb1 = allgather(ag_t2)
                # hop1 (streams y1T, ready at u1 end): runs under AG1;
                # the hop0/hop1 half of the BN AllReduce also fires here
                # (collective cores are idle after AG1)
                with tier():
                    for g in range(NG):
                        hop12_grp(1, g)
                    A1 = stpool.tile([P, 3], f32, name="A1")
                    B1 = stpool.tile([P, 3], f32, name="B1")
                    g1a = bn_ar(0, [(st[(1, "S0")], st[(1, "Q0")]),
                                    (st[(1, "S1")], st[(1, "Q1")])])
                    ab_cols(g1a, 0, [0, 1], A1, B1, "1a")
                # u2: A.(A x) -> y2T
                with tier():
                    spmm_unit(tb1[:], mk_sink_hT(y2T), "u2")
                # hop2: same phase as u2 - scheduler interleaves each
                # group as its y2T tiles land
                for g in range(NG):
                    hop12_grp(2, g)


                # ============================================== LAYERS 2,3
                def relu_blocks(l, A, B, n0, w, tag):
                    hb = []
                    for b in range(3):
                        raw = hpool.tile([P, GSZ], bf16, name="raw",
                                         tag=f"raw{tag}{b}")
                        nc.sync.dma_start(raw[:, :w],
                                          rawh[(l, b)][:, n0:n0 + w])
                        h = hpool.tile([P, GSZ], bf16, name="hh",
                                       tag=f"h{tag}{b}")
                        nc.scalar.activation(out=h[:, :w], in_=raw[:, :w],
                                             func=AF.Relu,
                                             bias=B[:, b:b + 1],
                                             scale=A[:, b:b + 1])
                        hb.append(h)
                    return hb

                def relu_one(l, b, A, B, n0, w, tag):
                    raw = hpool.tile([P, GSZ], bf16, name="raw",
                                     tag=f"raw{tag}{b}")
                    nc.sync.dma_start(raw[:, :w], rawh[(l, b)][:, n0:n0 + w])
                    h = hpool.tile([P, GSZ], bf16, name="hh",
                                   tag=f"h{tag}{b}")
                    nc.scalar.activation(out=h[:, :w], in_=raw[:, :w],
                                         func=AF.Relu, bias=B[:, b:b + 1],
                                         scale=A[:, b:b + 1])
                    return h

                def tf_sweep_j2(l, A, B):
                    """t2 = sum_b W[l][2,b] h_b + bias -> ag_t2 (node-major)"""
                    for grp in range(NG):
                        n0 = grp * GSZ
                        w = min(GSZ, SHARD - n0)
                        hb = relu_blocks(l, A, B, n0, w, "a")
                        nm_tblock(l, 2, hb, n0, w, ag_t2)

                def tf_sweep_j2_partial(l, A, B):
                    """partial t2 = W[l][2,0] h0 + W[l][2,1] h1 (no bias, no
                    dinv) -> part_d, node-major; hidden under the previous
                    spmm pass (needs only hop0/hop1 A/B from the early AR)."""
                    for grp in range(NG):
                        n0 = grp * GSZ
                        w = min(GSZ, SHARD - n0)
                        h0 = relu_one(l, 0, A, B, n0, w, "a")
                        h1 = relu_one(l, 1, A, B, n0, w, "a")
                        ps = ps_tf.tile([P, GSZ], f32, name="nmp", tag="tf",
                                        space="PSUM")
                        for q in range(w // P):
                            c0 = q * P
                            nc.tensor.matmul(ps[:, c0:c0 + P],
                                             lhsT=h0[:, c0:c0 + P],
                                             rhs=wblk(l, 2, 0),
                                             start=True, stop=False)
                            nc.tensor.matmul(ps[:, c0:c0 + P],
                                             lhsT=h1[:, c0:c0 + P],
                                             rhs=wblk(l, 2, 1),
                                             start=False, stop=True)
                        for q in range(w // P):
                            c0 = q * P
                            ob = wpool.tile([P, P], bf16, name="obp",
                                            tag="ob", bufs=6)
                            nc.scalar.activation(out=ob[:],
                                                 in_=ps[:, c0:c0 + P],
                                                 func=AF.Copy)
                            nc.sync.dma_start(
                                part_d[n0 + c0:n0 + c0 + P, :], ob[:])

                def tf_sweep_j2_final(l, A, B):
                    """t2 = partial + W[l][2,2] h2 + bias, dinv-scaled ->
                    ag_t2.  The partial is added back into the PSUM chain
                    with an identity-lhsT matmul (stays on PE)."""
                    ones = rbias_sb[:1, 9 * P:10 * P]
                    for grp in range(NG):
                        n0 = grp * GSZ
                        w = min(GSZ, SHARD - n0)
                        h2 = relu_one(l, 2, A, B, n0, w, "a")
                        ps = ps_tf.tile([P, GSZ], f32, name="nmf", tag="tf",
                                        space="PSUM")
                        pl = []
                        for q in range(w // P):
                            c0 = q * P
                            plq = wpool.tile([P, P], bf16, name="pl",
                                             tag="pl", bufs=4)
                            nc.sync.dma_start(
                                plq[:], part_d[n0 + c0:n0 + c0 + P, :])
                            pl.append(plq)
                        for q in range(w // P):
                            c0 = q * P
                            nc.tensor.matmul(ps[:, c0:c0 + P],
                                             lhsT=h2[:, c0:c0 + P],
                                             rhs=wblk(l, 2, 2),
                                             start=True, stop=False)
                            nc.tensor.matmul(ps[:, c0:c0 + P],
                                             lhsT=ident_sb, rhs=pl[q][:],
                                             start=False, stop=False)
                            nc.tensor.matmul(ps[:, c0:c0 + P], lhsT=ones,
                                             rhs=brow(l, 2), start=False,
                                             stop=True)
                        for q in range(w // P):
                            c0 = q * P
                            tt = (n0 + c0) // P
                            ob = wpool.tile([P, P], bf16, name="obf",
                                            tag="ob", bufs=6)
                            nc.scalar.activation(out=ob[:],
                                                 in_=ps[:, c0:c0 + P],
                                                 func=AF.Copy,
                                                 scale=dinv1_sb[:, tt:tt + 1])
                            nc.sync.dma_start(
                                ag_t2[n0 + c0:n0 + c0 + P, :], ob[:])

                def tf_sweep_j01(l, A, B, final):
                    """j=0 (stats+rawh or final out) and j=1 (-> ag_t1)."""
                    for grp in range(NG):
                        n0 = grp * GSZ
                        w = min(GSZ, SHARD - n0)
                        hb = relu_blocks(l, A, B, n0, w, "b")
                        if final:
                            nm_out0(l, hb, n0, w)
                        else:
                            ps = ps_tf.tile([P, GSZ], f32, name="tfj0",
                                            tag="tf", space="PSUM")
                            for b in range(3):
                                nc.tensor.matmul(ps[:, :w], lhsT=wblk(l, 0, b),
                                                 rhs=hb[b][:, :w],
                                                 start=(b == 0), stop=(b == 2))
                            tf_stats_raw(ps, w, grp, st[(2, "S0")],
                                         st[(2, "Q0")], rawh[(2, 0)], n0,
                                         bias=bcol(l, 0))
                        nm_tblock(l, 1, hb, n0, w, ag_t1)

                # ---------- layer 2
                for key in ("S0", "Q0", "S1", "Q1", "S2", "Q2"):
                    ncols = NG if key in ("S0", "Q0") else TPC
                    st[(2, key)] = stpool.tile([P, ncols], f32, name=f"st2{key}")

                with tier():
                    g1b = bn_ar(1, [(st[(1, "S2")], st[(1, "Q2")])])
                    ab_cols(g1b, 0, [2], A1, B1, "1b")
                    tf_sweep_j2(1, A1, B1)
                    tb_t2 = allgather(ag_t2)       # starts as soon as t2 done
                with tier():
                    tf_sweep_j01(1, A1, B1, final=False)   # under AG(t2)
                    tb_t1 = allgather(ag_t1)
                with tier():
                    # p1: A.t2 -> us gather table only
                    spmm_unit(tb_t2[:], mk_sink_tblwr(ag_us, 2), "v1")
                    tb_us = allgather(ag_us)
                with tier():
                    # p2: A.t1 -> hop1 block (runs under AG(us)); early
                    # half of the layer-2 BN AllReduce fires right after
                    A2 = stpool.tile([P, 3], f32, name="A2")
                    B2 = stpool.tile([P, 3], f32, name="B2")
                    spmm_unit(tb_t1[:],
                              mk_sink_block(2, 1, st[(2, "S1")],
                                            st[(2, "Q1")]),
                              "v2")
                    g2a = bn_ar(2, [(st[(2, "S0")], st[(2, "Q0")]),
                                    (st[(2, "S1")], st[(2, "Q1")])])
                    ab_cols(g2a, 1, [0, 1], A2, B2, "2a")
                with tier():
                    # p3: A.us -> hop2 block; ARa (hop0/hop1 stats) and the
                    # partial t2 sweep for layer 3 hide under it.
                    spmm_unit(tb_us[:],
                              mk_sink_block(2, 2, st[(2, "S2")],
                                            st[(2, "Q2")]),
                              "v3")

                # ---------- layer 3 (final: no BN on outputs)
                with tier():
                    g2b = bn_ar(3, [(st[(2, "S2")], st[(2, "Q2")])])
                    ab_cols(g2b, 1, [2], A2, B2, "2b")
                    tf_sweep_j2(2, A2, B2)
                    tb_t2b = allgather(ag_t2)
                with tier():
                    tf_sweep_j01(2, A2, B2, final=True)  # j0 -> OUT cols 0:P
                    tb_t1b = allgather(ag_t1)
                with tier():
                    spmm_unit(tb_t2b[:], mk_sink_tblwr(ag_us, 2), "w1")
                    tb_usb = allgather(ag_us)
                with tier():
                    spmm_unit(tb_t1b[:], mk_sink_out(P), "w2")
                with tier():
                    spmm_unit(tb_usb[:], mk_sink_out(2 * P), "w3")

    if not for_sim:
        _split_excess_waits(nc)
        mybir.codegen_inst_isa_subclasses(nc)
    return nc


_CACHE = {}


def kernel(x, edge_index, W0, b0, W1, b1, W2, b2, bn_gamma, bn_beta):
    x = np.asarray(x, np.float32)
    edge_index = np.asarray(edge_index)
    xtbl, cores, K5 = _host_prep(x, edge_index)

    W0 = np.asarray(W0, np.float32)
    W1 = np.asarray(W1, np.float32)
    W2 = np.asarray(W2, np.float32)
    b0 = np.asarray(b0, np.float32)
    b1 = np.asarray(b1, np.float32)
    b2 = np.asarray(b2, np.float32)
    bn_g = np.asarray(bn_gamma, np.float32)
    bn_b = np.asarray(bn_beta, np.float32)

    in_maps = []
    for c in range(NC):
        cb, cr, cf = _pack_consts(cores[c], K5, W0, b0, W1, b1, W2, b2,
                                  bn_g, bn_b)
        in_maps.append(dict(
            xtbl=xtbl, xt=cores[c]["xt"], idx=cores[c]["idx16"],
            cb=cb, cr=cr, cf=cf))

    phase = os.environ.get("KPHASE", "full")
    if (K5, phase) not in _CACHE:
        _CACHE[(K5, phase)] = _build(K5, phase)
    nc = _CACHE[(K5, phase)]
    trace = bool(os.environ.get("KERNEL_TRACE"))
    res = run_bass_kernel_spmd(nc, in_maps, core_ids=list(range(NC)),
                               trace=trace)
    global last_result
    last_result = res
    out = np.concatenate([r["out"] for r in res.results], axis=0)
    return out[:N].astype(np.float32)


last_result = None

